# revision 2
# baseline (speedup 1.0000x reference)
"""BitNet-style quantized MLP (nn_ExpertMLP) on 8 Trainium2 NeuronCores.

Math (per reference):
    h = silu(Qa(x) @ Qw(W1).T);  y = Qa(h) @ Qw(W2).T
    Qa: per-token int8 absmax quant  -> round(x * 127/clip(max|x|,1e-5)) / s
    Qw: per-tensor ternary quant     -> clip(round(w / clip(mean|w|,1e-5)), -1, 1) * mean

Strategy: pure data parallel over tokens (2048 tokens/core, no collectives).
Quantized values are small integers, so matmuls run EXACTLY as bf16(acts) x
fp8e4(ternary weights) with fp32 PSUM accumulation; per-token/per-tensor
scales are folded into the output epilogues.

v2: software-pipelined phase A. Per token tile the per-engine program order is
  [MMs(tt) | x-prep(tt+1) | h-quant+spill(tt-1) | silu+stats(tt) | W2-prestage]
so DVE/ACT/DMA epilogue+prep work runs under the next tile's matmul stream
instead of serializing after it (baseline lost ~92us/tile to this). Tile 0's
matmuls are interleaved into the W1 load+quantize stream (fc-major). The h
spill keeps the DMA-transpose's native [t-block, f-block] tiling in DRAM so
both spill and phase-B reload are fully contiguous per partition; phase B
prefetches the first h slab before the W2 resident load and double-buffers
the rest under the matmuls.
"""
import numpy as np

import bass_rust
import concourse.bass as bass
import concourse.tile as tile
from concourse import mybir
from concourse.bass_utils import run_bass_kernel_spmd
from concourse.vector_clock import ScopedClock

D_MODEL = 2048
D_FF = 8192
N_CORES = 8
T_TOTAL = 4 * 4096
T_CORE = T_TOTAL // N_CORES          # 2048 tokens per core
N_TT = T_CORE // 128                 # 16 token tiles per core
N_DT = D_MODEL // 128                # 16 contraction tiles (layer 1)
N_FC = D_FF // 512                   # 16 f-chunks of 512
N_FT = D_FF // 128                   # 64 contraction tiles (layer 2)
N_MC = D_MODEL // 512                # 4 output chunks (layer 2)

MAGIC = 12582912.0                   # 1.5 * 2**23: (x + MAGIC) - MAGIC == RNE round(x)
F32 = mybir.dt.float32
BF16 = mybir.dt.bfloat16
FP8 = mybir.dt.float8e4

ANNOT = {}          # instruction name -> human label (debug/profiling aid)


def _ann(res, label):
    try:
        ANNOT[res.ins.name] = label
    except AttributeError:
        pass
    return res

# ---------------------------------------------------------------------------
# walrus in this container rejects instructions carrying >1 sem wait
# ("Too many sync wait commands"); split excess waits onto same-engine NOPs.
MAXW = 1


def _split_one(nc, bb, inst):
    si = inst.sync_info
    waits = list(si.on_wait) if si and si.on_wait else []
    if len(waits) <= MAXW:
        return
    keep, extra = waits[-MAXW:], waits[:-MAXW]
    inst.sync_info = bass_rust.SyncInfo(on_wait=keep, on_update=list(si.on_update or []))
    eng = nc.engines[inst.engine]
    nops = []
    for i in range(0, len(extra), MAXW):
        n = eng.nop()
        n.ins.sync_info = bass_rust.SyncInfo(on_wait=extra[i:i + MAXW], on_update=[])
        nops.append(n.ins)
    cur = nc.cur_bb.bb
    cur_insts = cur.instructions
    for n in nops:
        for j in range(len(cur_insts) - 1, -1, -1):
            if cur_insts[j].name == n.name:
                cur_insts.pop(j)
                break
    cur.instructions = cur_insts
    insts = bb.instructions
    for j, x in enumerate(insts):
        if x.name == inst.name:
            for k, n in enumerate(nops):
                insts.insert(j + k, n)
            break
    bb.instructions = insts


def split_waits(nc):
    for _, bass_bb in list(nc.bb_map.items()):
        bb = bass_bb.bb
        for inst in list(bb.instructions):
            si = inst.sync_info
            if si and si.on_wait and len(si.on_wait) > MAXW:
                _split_one(nc, bb, inst)


class SplitDrainTC(tile.TileContext):
    def _drain_and_barrier(self, tick_clock, wait_clock):
        nc = self.nc
        probe = nc.sync.nop()
        wait_clock.add_sem_waits(probe.ins, ScopedClock({None: tick_clock.global_clock}))
        si = probe.ins.sync_info
        waits = list(si.on_wait) if si and si.on_wait else []
        if len(waits) > MAXW:
            probe.ins.sync_info = bass_rust.SyncInfo(
                on_wait=waits[:MAXW], on_update=list(si.on_update or []))
            for i in range(MAXW, len(waits), MAXW):
                n2 = nc.sync.nop()
                n2.ins.sync_info = bass_rust.SyncInfo(on_wait=waits[i:i + MAXW], on_update=[])
        nc.sync.drain()
        nc.all_engine_barrier()
        popped = nc._tile_sem_poison_stack.pop()
        assert popped is self._sem_poison
        nc.clear_and_free_semaphores(list(self.sems.allocated().values()))
        nc.all_engine_barrier()


# ---------------------------------------------------------------------------


def _build_nc():
    nc = bass.Bass()
    x_in = nc.declare_dram_parameter("x", [T_CORE, D_MODEL], F32, isOutput=False)
    w1t = nc.declare_dram_parameter("w1t", [D_MODEL, D_FF], F32, isOutput=False)
    w2t = nc.declare_dram_parameter("w2t", [D_FF, D_MODEL], F32, isOutput=False)
    # [s_w1, s_w2, a1mul=clip(mean|W1|)/127, a2mul=clip(mean|W2|)/127]
    consts = nc.declare_dram_parameter("consts", [1, 4], F32, isOutput=False)
    y_out = nc.declare_dram_parameter("y", [T_CORE, D_MODEL], F32, isOutput=True)

    AF = mybir.ActivationFunctionType

    with SplitDrainTC(nc) as tc:
        with tc.tile_pool(name="persist", bufs=1) as persist:
            csb = persist.tile([128, 4], F32, tag="consts")
            nc.sync.dma_start(out=csb, in_=consts[0:1, :].to_broadcast((128, 4)))
            s_w1 = csb[:, 0:1]
            s_w2 = csb[:, 1:2]
            a1mul = csb[:, 2:3]
            a2mul = csb[:, 3:4]
            magic = persist.tile([128, 1], F32, tag="magic")
            nc.vector.memset(magic, MAGIC)
            # per-token-tile scales, one column per tile
            alpha2 = persist.tile([128, N_TT], F32, tag="alpha2")
            al1s = persist.tile([128, N_TT], F32, tag="al1s")
            s1s = persist.tile([128, N_TT], F32, tag="s1s")
            s2s = persist.tile([128, N_TT], F32, tag="s2s")

            with tc.tile_pool(name="hspill", bufs=1, space="DRAM") as dpool:
                # h spill keeps the transpose's native tiling: [t-part, fc, b, t]
                hsp = [dpool.tile([128, N_FC, 4, 128], BF16, tag=f"hsp{tt}", name=f"hsp_{tt}")
                       for tt in range(N_TT)]
                w2d = [dpool.tile([128, 4, D_MODEL], FP8, tag=f"w2d{g}", name=f"w2d_{g}")
                       for g in range(N_FT // 4)]

                # ============================= PHASE A =============================
                with tc.tile_pool(name="w1res", bufs=1) as w1pool:
                    w1q = [w1pool.tile([128, D_FF], FP8, tag=f"w1_{d}", name=f"w1q_{d}")
                           for d in range(N_DT)]

                    with tc.tile_pool(name="am", bufs=1) as am, \
                         tc.tile_pool(name="psA", bufs=8, space="PSUM") as psA:

                        xqT = {}
                        xsts = {}

                        def prep_load(tt):
                            trow = slice(tt * 128, (tt + 1) * 128)
                            xst = am.tile([128, D_MODEL], F32, tag="xst", bufs=1,
                                          name=f"xst_{tt}")
                            xsts[tt] = xst
                            nc.scalar.dma_start(out=xst, in_=x_in[trow, :])

                        def prep(tt):
                            """per-token absmax + quantize + transpose."""
                            xst = xsts.pop(tt)
                            cmax = am.tile([128, 4], F32, tag="cmax", bufs=2)
                            for c in range(4):
                                nc.vector.tensor_reduce(
                                    out=cmax[:, c:c + 1], in_=xst[:, c * 512:(c + 1) * 512],
                                    axis=mybir.AxisListType.X,
                                    op=mybir.AluOpType.max, apply_absolute_value=True)
                            amax = am.tile([128, 1], F32, tag="amax", bufs=2)
                            nc.vector.tensor_reduce(out=amax, in_=cmax,
                                                    axis=mybir.AxisListType.X,
                                                    op=mybir.AluOpType.max)
                            nc.vector.tensor_scalar_max(amax, amax, 1e-5)
                            s1 = s1s[:, tt:tt + 1]
                            nc.vector.reciprocal(s1, amax)
                            nc.vector.tensor_scalar_mul(s1, s1, 127.0)
                            nc.vector.tensor_scalar(out=al1s[:, tt:tt + 1], in0=amax,
                                                    scalar1=a1mul, scalar2=None,
                                                    op0=mybir.AluOpType.mult)
                            xq = am.tile([128, N_DT, 128], BF16, tag="xqT", bufs=2,
                                         name=f"xqT_{tt}")
                            xqT[tt] = xq
                            for c in range(4):
                                t1 = am.tile([128, 512], F32, tag="qt", bufs=2)
                                nc.vector.tensor_scalar(
                                    out=t1, in0=xst[:, c * 512:(c + 1) * 512],
                                    scalar1=s1, scalar2=MAGIC,
                                    op0=mybir.AluOpType.mult, op1=mybir.AluOpType.add)
                                xqc = am.tile([128, 512], BF16, tag="xqc", bufs=2)
                                nc.vector.tensor_scalar_add(xqc, t1, -MAGIC)
                                _ann(nc.scalar.dma_start_transpose(
                                    xq[:, c * 4:(c + 1) * 4, :], xqc), f"xqtr[{tt},{c}]")

                        pssA = {}

                        def mm_blk(tt, blk):
                            # blk = 2 fc-chunks = 2 PSUM banks; 4 blocks in flight
                            pss = []
                            for i in range(2):
                                ps = psA.tile([128, 512], F32, tag="psA",
                                              name=f"psA_{tt}_{blk}_{i}")
                                pss.append(ps)
                            pssA[(tt, blk)] = pss
                            xq = xqT[tt]
                            for d in range(N_DT):
                                for i in range(2):
                                    fc = blk * 2 + i
                                    _ann(nc.tensor.matmul(
                                        pss[i], lhsT=xq[:, d, :],
                                        rhs=w1q[d][:, fc * 512:(fc + 1) * 512],
                                        start=(d == 0), stop=(d == N_DT - 1)),
                                        f"mmA[{tt},{blk},{d},{i}]")

                        hch = {}

                        def silu_blk(tt, blk):
                            """silu from PSUM -> bf16 hch halves (ACT only, prompt)."""
                            hc = am.tile([128, 1024], BF16, tag="h", bufs=8,
                                         name=f"h_{tt}_{blk}")
                            hch[(tt, blk)] = hc
                            for i in range(2):
                                fc = blk * 2 + i
                                _ann(nc.scalar.activation(
                                    out=hc[:, i * 512:(i + 1) * 512],
                                    in_=pssA[(tt, blk)][i], func=AF.Silu,
                                    scale=al1s[:, tt:tt + 1]),
                                     f"silu[{tt},{fc}]")

                        def hm_stats(tt):
                            """per-token absmax over h -> s2, alpha2 (DVE)."""
                            hmcol = am.tile([128, 8], F32, tag="hmcol", bufs=2,
                                            name=f"hmcol_{tt}")
                            for b in range(8):
                                nc.vector.tensor_reduce(
                                    out=hmcol[:, b:b + 1], in_=hch[(tt, b)],
                                    axis=mybir.AxisListType.X,
                                    op=mybir.AluOpType.max, apply_absolute_value=True)
                            for b in range(8):
                                del pssA[(tt, b)]
                            mh = am.tile([128, 1], F32, tag="mh", bufs=2)
                            nc.vector.tensor_reduce(out=mh, in_=hmcol,
                                                    axis=mybir.AxisListType.X,
                                                    op=mybir.AluOpType.max)
                            nc.vector.tensor_scalar_max(mh, mh, 1e-5)
                            s2 = s2s[:, tt:tt + 1]
                            nc.vector.reciprocal(s2, mh)
                            nc.vector.tensor_scalar_mul(s2, s2, 127.0)
                            nc.vector.tensor_scalar(out=alpha2[:, tt:tt + 1], in0=mh,
                                                    scalar1=a2mul, scalar2=None,
                                                    op0=mybir.AluOpType.mult)

                        def hq_round1(tt, b):
                            """round(h*s2) -> bf16, transpose, spill to DRAM."""
                            s2 = s2s[:, tt:tt + 1]
                            t2 = am.tile([128, 1024], F32, tag="qt2", bufs=2)
                            _ann(nc.vector.tensor_scalar(
                                out=t2, in0=hch[(tt, b)], scalar1=s2, scalar2=MAGIC,
                                op0=mybir.AluOpType.mult, op1=mybir.AluOpType.add),
                                 f"hqround[{tt},{b}]")
                            hqc = am.tile([128, 1024], BF16, tag="hqc", bufs=2)
                            nc.vector.tensor_scalar_add(hqc, t2, -MAGIC)
                            hqtr = am.tile([128, 2, 4, 128], BF16, tag="hqtr", bufs=2)
                            for i in range(2):
                                fc = b * 2 + i
                                _ann(nc.scalar.dma_start_transpose(
                                    hqtr[:, i], hqc[:, i * 512:(i + 1) * 512]), f"hqtr[{tt},{fc}]")
                            _ann(nc.scalar.dma_start(out=hsp[tt][:, b * 2:(b + 1) * 2], in_=hqtr),
                                 f"hspill[{tt},{b}]")
                            del hch[(tt, b)]

                        def hq_spill(tt):
                            for b in range(8):
                                hq_round1(tt, b)

                        w2sts = {}
                        w2rows = {}

                        def w2stage_load(tt):
                            for j in range(8):
                                f2, mc2 = divmod(tt * 8 + j, 2)
                                ms2 = slice(mc2 * 1024, (mc2 + 1) * 1024)
                                st2 = am.tile([128, 1024], F32, tag="w2st", bufs=2)
                                w2sts[(tt, j)] = st2
                                nc.sync.dma_start(
                                    out=st2, in_=w2t[f2 * 128:(f2 + 1) * 128, ms2])

                        def w2chunk(tt, j):
                            f2, mc2 = divmod(tt * 8 + j, 2)
                            ms2 = slice(mc2 * 1024, (mc2 + 1) * 1024)
                            st2 = w2sts.pop((tt, j))
                            t2 = am.tile([128, 1024], F32, tag="qt2", bufs=2)
                            _ann(nc.scalar.activation(out=t2, in_=st2, func=AF.Identity,
                                                 bias=magic, scale=s_w2),
                                 f"w2round[{tt},{j}]")
                            nc.vector.tensor_scalar(
                                out=t2, in0=t2, scalar1=-MAGIC, scalar2=1.0,
                                op0=mybir.AluOpType.add, op1=mybir.AluOpType.min)
                            if mc2 == 0:
                                w2rows[f2] = am.tile([128, D_MODEL], FP8, tag="w2c8",
                                                     bufs=2, name=f"w2row_{f2}")
                            c8 = w2rows[f2]
                            nc.vector.tensor_scalar(
                                out=c8[:, ms2], in0=t2, scalar1=-1.0, scalar2=None,
                                op0=mybir.AluOpType.max)
                            if mc2 == 1:
                                g, k = divmod(f2, 4)
                                _ann(nc.sync.dma_start(out=w2d[g][:, k], in_=w2rows.pop(f2)),
                                     f"w2store[{tt},{f2}]")

                        def w2stage(tt):
                            """quantize 1/16th of W2 -> fp8 in DRAM (for phase B)."""
                            for j in range(8):
                                w2chunk(tt, j)

                        # ---- head: x prep for tiles 0/1, W1 stream with tile-0
                        # matmuls interleaved (fc-major: blk k ready after fc 4k+3)
                        prep_load(0)
                        prep_load(1)
                        prep(0)
                        prep(1)
                        w2stage_load(0)
                        for b2 in range(8):
                            fs = slice(b2 * 1024, (b2 + 1) * 1024)
                            for d in range(N_DT):
                                st = am.tile([128, 1024], F32, tag="w1st", bufs=3)
                                nc.sync.dma_start(out=st, in_=w1t[d * 128:(d + 1) * 128, fs])
                                t1 = am.tile([128, 1024], F32, tag="qt2", bufs=2)
                                nc.scalar.activation(out=t1, in_=st, func=AF.Identity,
                                                     bias=magic, scale=s_w1)
                                nc.vector.tensor_scalar(
                                    out=t1, in0=t1, scalar1=-MAGIC, scalar2=1.0,
                                    op0=mybir.AluOpType.add, op1=mybir.AluOpType.min)
                                nc.vector.tensor_scalar(
                                    out=w1q[d][:, fs], in0=t1, scalar1=-1.0, scalar2=None,
                                    op0=mybir.AluOpType.max)
                            mm_blk(0, b2)
                            if b2 >= 1:
                                silu_blk(0, b2 - 1)
                        silu_blk(0, 7)
                        w2stage(0)
                        hm_stats(0)

                        # ---- steady state
                        for tt in range(1, N_TT):
                            if tt + 1 < N_TT:
                                prep_load(tt + 1)
                            w2stage_load(tt)
                            for blk in range(8):
                                mm_blk(tt, blk)
                                silu_blk(tt, blk)
                            hq_spill(tt - 1)
                            if tt + 1 < N_TT:
                                prep(tt + 1)
                            hm_stats(tt)
                            w2stage(tt)
                        hq_spill(N_TT - 1)

                # ============================= PHASE B =============================
                with tc.tile_pool(name="w2res", bufs=1) as w2pool, \
                     tc.tile_pool(name="bm", bufs=1) as bm, \
                     tc.tile_pool(name="psB", bufs=8, space="PSUM") as psB:
                    # first h slab on the scalar (ACT) HWDGE ring so it lands in
                    # parallel with the w2q stream on the sync (SP) ring
                    hslabs = {}

                    def load_hslab(tt):
                        hs = bm.tile([128, N_FC, 4, 128], BF16, tag="hslab", bufs=2,
                                     name=f"hslab_{tt}")
                        hslabs[tt] = hs
                        nc.scalar.dma_start(out=hs, in_=hsp[tt][:])
                        return hs

                    load_hslab(0)
                    w2q4 = [w2pool.tile([128, 4, D_MODEL], FP8, tag=f"w2_{g}", name=f"w2q_{g}")
                            for g in range(N_FT // 4)]
                    for g in range(N_FT // 4):
                        nc.sync.dma_start(out=w2q4[g], in_=w2d[g][:])
                    load_hslab(1)

                    for tt in range(N_TT):
                        trow = slice(tt * 128, (tt + 1) * 128)
                        hslab = hslabs[tt]
                        pss = []
                        for mc in range(N_MC):
                            ps = psB.tile([128, 512], F32, tag="psB",
                                          name=f"psB_{tt}_{mc}")
                            pss.append(ps)
                        for f in range(N_FT):
                            fc, b = divmod(f, 4)
                            g, k = divmod(f, 4)
                            for mc in range(N_MC):
                                _ann(nc.tensor.matmul(
                                    pss[mc], lhsT=hslab[:, fc, b, :],
                                    rhs=w2q4[g][:, k, mc * 512:(mc + 1) * 512],
                                    start=(f == 0), stop=(f == N_FT - 1)),
                                    f"mmB[{tt},{f},{mc}]")
                        if tt + 2 < N_TT:
                            load_hslab(tt + 2)
                        for mc in range(N_MC):
                            yc = bm.tile([128, 512], F32, tag="yc", bufs=4)
                            nc.scalar.activation(out=yc, in_=pss[mc], func=AF.Copy,
                                                 scale=alpha2[:, tt:tt + 1])
                            nc.sync.dma_start(
                                out=y_out[trow, mc * 512:(mc + 1) * 512], in_=yc)
                        del hslabs[tt]

    split_waits(nc)
    return nc


_NC_CACHE = None


def _get_nc():
    global _NC_CACHE
    if _NC_CACHE is None:
        _NC_CACHE = _build_nc()
    return _NC_CACHE


def kernel(x, W1, W2):
    assert x.shape == (4, 4096, D_MODEL) and x.dtype == np.float32
    assert W1.shape == (D_FF, D_MODEL) and W2.shape == (D_MODEL, D_FF)

    x2d = np.ascontiguousarray(x.reshape(T_TOTAL, D_MODEL))
    w1t = np.ascontiguousarray(W1.T)            # [D_MODEL, D_FF]
    w2t = np.ascontiguousarray(W2.T)            # [D_FF, D_MODEL]

    m1 = max(float(np.mean(np.abs(W1), dtype=np.float32)), 1e-5)
    m2 = max(float(np.mean(np.abs(W2), dtype=np.float32)), 1e-5)
    consts = np.array([[1.0 / m1, 1.0 / m2, m1 / 127.0, m2 / 127.0]], dtype=np.float32)

    nc = _get_nc()
    in_maps = [
        {"x": x2d[c * T_CORE:(c + 1) * T_CORE], "w1t": w1t, "w2t": w2t, "consts": consts}
        for c in range(N_CORES)
    ]
    res = run_bass_kernel_spmd(nc, in_maps, list(range(N_CORES)), trace=False)
    y = np.concatenate([res.results[c]["y"] for c in range(N_CORES)], axis=0)
    return y.reshape(4, 4096, D_MODEL)
